# revision 17
# baseline (speedup 1.0000x reference)
"""Trainium2 Bass kernel for DTWFeatures.

Problem: x (64,3,1024), patts (32,3,32) -> out (64,32,1024)
  dist[b,p,l,t] = sqrt(max(|x[b,:,t]-patts[p,:,l]|^2, eps))
  DP:  D[l,t] = dist[l,t] + min(D[l-1,t], w*D[l,t-1], w*D[l-1,t-1])
  out[b,p,t] = D[L-1,t]

Strategy (8 cores, data-parallel over batch, 8 batches/core, 256 (b,p)
pairs/core as 2 half-groups of 128 partitions):
  * Rescale E[l,t] = D[l,t]*w^-(t-SHIFT), removing w from the recurrence:
        E[l,t] = d'[l,t] + min(E[l-1,t], E[l-1,t-1], E[l,t-1])
    d'[l,t] = dist[l,t]*w^-(t-SHIFT).  SHIFT=512 keeps magnitudes in
    fp32/bf16 exponent range (E in ~[5e-19, 3e19]).
  * Per row l: ONE DVE tensor_tensor_scan (op0=min, op1=add) over BOTH
    half-groups packed along the free dim (2048 elems + INF edge slots):
        state_t = min(c_t, state_{t-1}) + d'_t,
        c_t = min(E[l-1,t], E[l-1,t-1])   (window-min of prev row)
    State crossing the h0->h1 boundary is harmless: E magnitudes at
    t=1023 (~1e19) exceed any h1 c_t (~1e-13) by >25 orders, so the min
    always picks the correct operand.
  * E buffers and the window-min are bfloat16: TensorTensor min runs in
    DVE 2x_1p mode (2-byte packed operands) at half cost.  The scan
    keeps fp32 internal state; only stored E values round to bf16
    (measured end-to-end L2 rel err ~3e-3 vs gate 2e-2).
  * dist'^2 comes from TensorE as K=17 float32r matmuls (1 cycle/row
    for free size >= 256, ~4x fp32): lhsT = [block-diag patts (12),
    per-b x2-indicators (4), p2+eps (1)], rhs = [x*(-2*w2inv) (12),
    x2*w2inv (4), w2inv (1)].  ScalarE sqrt PSUM->SBUF gives d'.
  * All x/patts-dependent tables are prepared on host (O(B*d*T) work);
    the device runs only DMAs, matmuls, sqrts, window-mins and scans.
    Final row is written fp32 and rescaled by w^(t-SHIFT) on host.
"""

import os
import sys

if "/opt/trn_rl_repo" not in sys.path:
    sys.path.insert(0, "/opt/trn_rl_repo")
# the device path runs through jax's axon PJRT backend; make sure a
# harness-pinned JAX_PLATFORMS doesn't hide it (no-op if jax is already up)
if "jax" not in sys.modules and "axon" not in os.environ.get(
    "JAX_PLATFORMS", "axon"
):
    os.environ["JAX_PLATFORMS"] = "axon," + os.environ["JAX_PLATFORMS"]

import numpy as np

NB, ND, NP, NL, NT = 64, 3, 32, 32, 1024   # batch, xdim, n_patts, l_patts, T
NCORES = 8
BPC = NB // NCORES                     # 8 batches per core
RHO = 0.1
W = RHO ** (1.0 / NL)
SHIFT = 512.0
EPS = 3e-3
INF = 1.0e30
K = 17                                 # matmul contraction rows

_CACHE = {}


def _tables():
    """Host-precomputed constant tables (x-independent parts)."""
    if "tables" not in _CACHE:
        t = np.arange(NT, dtype=np.float64)
        w2inv = (W ** (-2.0 * (t - SHIFT))).astype(np.float32)
        wpos = (W ** (t - SHIFT)).astype(np.float32)
        _CACHE["tables"] = (w2inv, wpos)
    return _CACHE["tables"]


def _build(debug=False):
    key = ("nc", debug)
    if key in _CACHE:
        return _CACHE[key]

    from contextlib import ExitStack

    import concourse.bass as bass  # noqa: F401
    import concourse.tile as tile
    from concourse import bacc, mybir

    f32 = mybir.dt.float32
    f32r = mybir.dt.float32r
    bf16 = mybir.dt.bfloat16
    AOT = mybir.AluOpType

    nc = bacc.Bacc(None, target_bir_lowering=False)
    lhst_d = nc.dram_tensor("lhst", [K, 128, NL], f32r, kind="ExternalInput")
    # gate = xw0 | xw1 | lhsT[:, :, 0:2] packed (m,j) -- one DMA covers
    # everything rows 0-1 need (HWDGE serializes at ~630ns per DMA)
    gate_d = nc.dram_tensor("gate", [K, 2 * NT + 256], f32r, kind="ExternalInput")
    out_d = nc.dram_tensor("eout", [2, 128, NT], bf16, kind="ExternalOutput")

    with tile.TileContext(nc) as tc:
        with ExitStack() as ctx:
            persist = ctx.enter_context(tc.tile_pool(name="persist", bufs=1))
            dist_pool = ctx.enter_context(tc.tile_pool(name="dist", bufs=3))
            c_pool = ctx.enter_context(tc.tile_pool(name="cmin", bufs=2))
            psum_pool = ctx.enter_context(
                tc.tile_pool(name="psum", bufs=2, space="PSUM")
            )

            lhsT = persist.tile([K, 128, NL], f32r, name="lhsT")
            gate = persist.tile([K, 2 * NT + 256], f32r, name="gate")
            xw = [gate[:, 0:NT], gate[:, NT : 2 * NT]]
            lhsT01 = gate[:, 2 * NT : 2 * NT + 256].rearrange(
                "k (m j) -> k m j", j=2
            )
            inf2 = persist.tile([128, NT], bf16, name="inf2")
            # E layout: slot 0 = INF edge, slots 1..2048 = E packed h0|h1.
            # No separator between halves: h0's t=1023 value (~1e19) can
            # never win a min against h1 values (~1e-13), so the wmin
            # window crossing the boundary is exact anyway.
            E0 = persist.tile([128, 2 * NT + 1], bf16, name="E0")
            E1 = persist.tile([128, 2 * NT + 1], bf16, name="E1")
            E = [E0, E1]
            Eout = persist.tile([128, 2 * NT], bf16, name="Eout")

            # ---------------- startup ----------------
            wsrc = persist.tile([1, 256], bf16, name="wsrc")
            nc.vector.memset(wsrc[:], 0.0)
            actd = persist.tile([1, 1], f32, name="actd")
            nc.vector.memset(actd[:], 1.0)
            nc.scalar.sqrt(actd[:], actd[:])  # preload the Sqrt ACT table
            # PE pstate warmup: keep TensorE busy from ~t=1us so the row-0
            # matmuls dispatch against a warm ramp instead of cold pstate
            nc.sync.dma_start(gate[:], gate_d[:])
            nc.scalar.dma_start(lhsT[:], lhst_d[:])
            nc.gpsimd.memset(inf2[:], INF)
            nc.vector.memset(E0[:, 0:1], INF)
            nc.vector.memset(E1[:, 0:1], INF)

            # ---------------- main loop over DP rows ----------------
            for j in range(NL):
                d3 = dist_pool.tile([128, 2 * NT], f32, name="d3")
                ps = psum_pool.tile([128, 2 * NT], f32, name="ps")
                if j == 0:
                    # PE pstate warmup: keep TensorE busy from ~t=1us so
                    # the row-0 matmuls dispatch against a warm ramp
                    for _ in range(11):
                        nc.tensor.matmul(
                            ps[0:1, 0:256], wsrc[:, 0:1], wsrc[:],
                            start=True, stop=True, skip_group_check=True,
                        )
                for hh in range(2):
                    for q in range(2):
                        c0 = hh * NT + q * (NT // 2)
                        nc.tensor.matmul(
                            ps[:, c0 : c0 + NT // 2],
                            lhsT01[:, :, j] if j < 2 else lhsT[:, :, j],
                            xw[hh][:, q * (NT // 2) : (q + 1) * (NT // 2)],
                            start=True,
                            stop=True,
                        )
                    # per-half d' sqrt: finer-grained deps (scan_h
                    # waits only its own half) and keeps the tc
                    # scheduler from hoisting a 2048-wide op between
                    # another row's halves.  Row 0: 512-wide quarters
                    # so the first scan chunk starts earlier.
                    if j == 0:
                        for q in range(2):
                            s0 = hh * NT + q * (NT // 2)
                            nc.scalar.sqrt(
                                d3[:, s0 : s0 + NT // 2],
                                ps[:, s0 : s0 + NT // 2],
                            )
                    else:
                        nc.scalar.sqrt(
                            d3[:, hh * NT : (hh + 1) * NT],
                            ps[:, hh * NT : (hh + 1) * NT],
                        )

                Ecur, Eprev = E[j % 2], E[(j + 1) % 2]
                if j == 0:
                    # cumsum per half (state resets to 0 at each half's
                    # start), chunked 512-wide to chase the quarter
                    # sqrts; chunk q>0 chains off the previous chunk's
                    # last output slot
                    HB = NT // 2
                    for hh in range(2):
                        for q in range(2):
                            s0 = hh * NT + q * HB
                            nc.vector.tensor_tensor_scan(
                                out=Ecur[:, s0 + 1 : s0 + HB + 1],
                                data0=inf2[:, 0:HB],
                                data1=d3[:, s0 : s0 + HB],
                                initial=(0.0 if q == 0
                                         else Ecur[:, s0 : s0 + 1]),
                                op0=AOT.min,
                                op1=AOT.add,
                            )
                    # row 0 is monotone in t: row 1's window-min is the
                    # shifted row.  Patch the two positions the shifted
                    # view gets wrong: slot 0 <- E_h0[0] (edge) and slot
                    # NT <- E_h1[0] (E_h0[NT-1] there is never needed:
                    # by monotonicity position NT-1 reads slot NT-1).
                    nc.vector.tensor_copy(
                        out=Ecur[:, 0:1], in_=Ecur[:, 1:2]
                    )
                    nc.vector.tensor_copy(
                        out=Ecur[:, NT : NT + 1], in_=Ecur[:, NT + 1 : NT + 2]
                    )
                elif j == 1:
                    # min(E0[t], E0[t-1]) == E0[t-1] by monotonicity:
                    # the shifted row IS the window-min, no TT op
                    for hh in range(2):
                        nc.vector.tensor_tensor_scan(
                            out=Ecur[:, hh * NT + 1 : (hh + 1) * NT + 1],
                            data0=Eprev[:, hh * NT : (hh + 1) * NT],
                            data1=d3[:, hh * NT : (hh + 1) * NT],
                            initial=INF,
                            op0=AOT.min,
                            op1=AOT.add,
                        )
                    # restore the INF edge for later rows reusing E0
                    nc.vector.memset(Eprev[:, 0:1], INF)
                else:
                    # Window-min in bf16 (2x_1p DVE mode, half cost),
                    # split per half-group and interleaved with the
                    # scans so every same-engine dependency has a full
                    # op of slack (no semaphore bubbles on DVE):
                    #   wmin_h0, wmin_h1, scan_h0, scan_h1
                    c3 = c_pool.tile([128, 2 * NT], bf16, name="c3")
                    for hh in range(2):
                        nc.vector.tensor_tensor(
                            c3[:, hh * NT : (hh + 1) * NT],
                            Eprev[:, hh * NT + 1 : (hh + 1) * NT + 1],
                            Eprev[:, hh * NT : (hh + 1) * NT],
                            op=AOT.min,
                        )
                    if j < NL - 1:
                        for hh in range(2):
                            nc.vector.tensor_tensor_scan(
                                out=Ecur[:, hh * NT + 1 : (hh + 1) * NT + 1],
                                data0=c3[:, hh * NT : (hh + 1) * NT],
                                data1=d3[:, hh * NT : (hh + 1) * NT],
                                initial=INF,
                                op0=AOT.min,
                                op1=AOT.add,
                            )
                    else:
                        # final row: chunked scans + eager DMAs so only
                        # the last (short) chunk's DMA sits in the tail
                        chunks = [(0, 0, 1024), (1, 0, 896), (1, 896, 128)]
                        for ci, (hh, q0, ln) in enumerate(chunks):
                            s0 = hh * NT + q0
                            nc.vector.tensor_tensor_scan(
                                out=Eout[:, s0 : s0 + ln],
                                data0=c3[:, s0 : s0 + ln],
                                data1=d3[:, s0 : s0 + ln],
                                initial=(INF if q0 == 0
                                         else Eout[:, s0 - 1 : s0]),
                                op0=AOT.min,
                                op1=AOT.add,
                            )
                            eng = (nc.sync, nc.scalar)[ci % 2]
                            eng.dma_start(
                                out_d[hh][:, q0 : q0 + ln],
                                Eout[:, s0 : s0 + ln],
                            )



    nc.compile()
    _CACHE[key] = nc
    return nc


def _in_maps(x, patts):
    w2inv, _ = _tables()
    x = np.asarray(x, dtype=np.float32)
    patts = np.asarray(patts, dtype=np.float32)

    # lhsT [K, 128, NL]: rows 0..11 block-diag patts (bq*3+d, bq*32+p),
    # rows 12..15 per-b x2 indicators, row 16 = p2 + eps
    lhst = np.zeros((K, 128, NL), np.float32)
    for bq in range(4):
        for d in range(ND):
            lhst[bq * 3 + d, bq * 32 : (bq + 1) * 32, :] = patts[:, d, :]
        lhst[12 + bq, bq * 32 : (bq + 1) * 32, :] = 1.0
    p2e = (patts ** 2).sum(axis=1) + EPS          # (P, NL)
    lhst[16, :, :] = np.tile(p2e, (4, 1))

    maps = []
    for c in range(NCORES):
        xb = x[c * BPC : (c + 1) * BPC]           # (8, 3, NT)
        x2 = (xb ** 2).sum(axis=1)                # (8, NT)
        xws = []
        for h in range(2):
            xwh = np.empty((K, NT), np.float32)
            for bq in range(4):
                b = h * 4 + bq
                xwh[bq * 3 : bq * 3 + 3] = xb[b] * (-2.0 * w2inv)[None, :]
                xwh[12 + bq] = x2[b] * w2inv
            xwh[16] = w2inv
            xws.append(np.ascontiguousarray(xwh))
        maps.append(
            {
                "lhst": np.ascontiguousarray(lhst),
                "gate": np.ascontiguousarray(
                    np.concatenate(
                        [xws[0], xws[1], lhst[:, :, 0:2].reshape(K, 256)],
                        axis=1,
                    )
                ),
            }
        )
    return maps


def _post(res):
    _, wpos = _tables()
    outs = []
    for r in res.results:
        e = np.asarray(r["eout"], dtype=np.float32)   # (2, 128, NT)
        outs.append(e.reshape(BPC, NP, NT) * wpos[None, None, :])
    return np.concatenate(outs, axis=0).astype(np.float32)


def kernel(x, patts):
    nc = _build()
    from concourse.bass_utils import run_bass_kernel_spmd

    res = run_bass_kernel_spmd(
        nc, _in_maps(x, patts), core_ids=list(range(NCORES))
    )
    _CACHE["last_results"] = res
    return _post(res)


# revision 18
# speedup vs baseline: 1.0141x; 1.0141x over previous
"""Trainium2 Bass kernel for DTWFeatures.

Problem: x (64,3,1024), patts (32,3,32) -> out (64,32,1024)
  dist[b,p,l,t] = sqrt(max(|x[b,:,t]-patts[p,:,l]|^2, eps))
  DP:  D[l,t] = dist[l,t] + min(D[l-1,t], w*D[l,t-1], w*D[l-1,t-1])
  out[b,p,t] = D[L-1,t]

Strategy (8 cores, data-parallel over batch, 8 batches/core, 256 (b,p)
pairs/core as 2 half-groups of 128 partitions):
  * Rescale E[l,t] = D[l,t]*w^-(t-SHIFT), removing w from the recurrence:
        E[l,t] = d'[l,t] + min(E[l-1,t], E[l-1,t-1], E[l,t-1])
    d'[l,t] = dist[l,t]*w^-(t-SHIFT).  SHIFT=512 keeps magnitudes in
    fp32/bf16 exponent range (E in ~[5e-19, 3e19]).
  * Per row l: ONE DVE tensor_tensor_scan (op0=min, op1=add) over BOTH
    half-groups packed along the free dim (2048 elems + INF edge slots):
        state_t = min(c_t, state_{t-1}) + d'_t,
        c_t = min(E[l-1,t], E[l-1,t-1])   (window-min of prev row)
    State crossing the h0->h1 boundary is harmless: E magnitudes at
    t=1023 (~1e19) exceed any h1 c_t (~1e-13) by >25 orders, so the min
    always picks the correct operand.
  * E buffers and the window-min are bfloat16: TensorTensor min runs in
    DVE 2x_1p mode (2-byte packed operands) at half cost.  The scan
    keeps fp32 internal state; only stored E values round to bf16
    (measured end-to-end L2 rel err ~3e-3 vs gate 2e-2).
  * dist'^2 comes from TensorE as K=17 float32r matmuls (1 cycle/row
    for free size >= 256, ~4x fp32): lhsT = [block-diag patts (12),
    per-b x2-indicators (4), p2+eps (1)], rhs = [x*(-2*w2inv) (12),
    x2*w2inv (4), w2inv (1)].  ScalarE sqrt PSUM->SBUF gives d'.
  * All x/patts-dependent tables are prepared on host (O(B*d*T) work);
    the device runs only DMAs, matmuls, sqrts, window-mins and scans.
    Final row is written fp32 and rescaled by w^(t-SHIFT) on host.
"""

import os
import sys

if "/opt/trn_rl_repo" not in sys.path:
    sys.path.insert(0, "/opt/trn_rl_repo")
# the device path runs through jax's axon PJRT backend; make sure a
# harness-pinned JAX_PLATFORMS doesn't hide it (no-op if jax is already up)
if "jax" not in sys.modules and "axon" not in os.environ.get(
    "JAX_PLATFORMS", "axon"
):
    os.environ["JAX_PLATFORMS"] = "axon," + os.environ["JAX_PLATFORMS"]

import numpy as np

NB, ND, NP, NL, NT = 64, 3, 32, 32, 1024   # batch, xdim, n_patts, l_patts, T
NCORES = 8
BPC = NB // NCORES                     # 8 batches per core
RHO = 0.1
W = RHO ** (1.0 / NL)
SHIFT = 512.0
EPS = 3e-3
INF = 1.0e30
K = 17                                 # matmul contraction rows

_CACHE = {}


def _tables():
    """Host-precomputed constant tables (x-independent parts)."""
    if "tables" not in _CACHE:
        t = np.arange(NT, dtype=np.float64)
        w2inv = (W ** (-2.0 * (t - SHIFT))).astype(np.float32)
        wpos = (W ** (t - SHIFT)).astype(np.float32)
        _CACHE["tables"] = (w2inv, wpos)
    return _CACHE["tables"]


def _build(debug=False):
    key = ("nc", debug)
    if key in _CACHE:
        return _CACHE[key]

    from contextlib import ExitStack

    import concourse.bass as bass  # noqa: F401
    import concourse.tile as tile
    from concourse import bacc, mybir

    f32 = mybir.dt.float32
    f32r = mybir.dt.float32r
    bf16 = mybir.dt.bfloat16
    AOT = mybir.AluOpType

    nc = bacc.Bacc(None, target_bir_lowering=False)
    lhst_d = nc.dram_tensor("lhst", [K, 128, NL], f32r, kind="ExternalInput")
    # gate = xw0 | xw1 | lhsT[:, :, 0:2] packed (m,j) -- one DMA covers
    # everything rows 0-1 need (HWDGE serializes at ~630ns per DMA)
    gate_d = nc.dram_tensor("gate", [K, 2 * NT + 256], f32r, kind="ExternalInput")
    out_d = nc.dram_tensor("eout", [2, 128, NT], bf16, kind="ExternalOutput")

    with tile.TileContext(nc) as tc:
        with ExitStack() as ctx:
            persist = ctx.enter_context(tc.tile_pool(name="persist", bufs=1))
            dist_pool = ctx.enter_context(tc.tile_pool(name="dist", bufs=3))
            c_pool = ctx.enter_context(tc.tile_pool(name="cmin", bufs=2))
            psum_pool = ctx.enter_context(
                tc.tile_pool(name="psum", bufs=2, space="PSUM")
            )

            lhsT = persist.tile([K, 128, NL], f32r, name="lhsT")
            gate = persist.tile([K, 2 * NT + 256], f32r, name="gate")
            xw = [gate[:, 0:NT], gate[:, NT : 2 * NT]]
            lhsT01 = gate[:, 2 * NT : 2 * NT + 256].rearrange(
                "k (m j) -> k m j", j=2
            )
            inf2 = persist.tile([128, NT], bf16, name="inf2")
            # E layout: slot 0 = INF edge, slots 1..2048 = E packed h0|h1.
            # No separator between halves: h0's t=1023 value (~1e19) can
            # never win a min against h1 values (~1e-13), so the wmin
            # window crossing the boundary is exact anyway.
            E0 = persist.tile([128, 2 * NT + 1], bf16, name="E0")
            E1 = persist.tile([128, 2 * NT + 1], bf16, name="E1")
            E = [E0, E1]
            Eout = persist.tile([128, 2 * NT], bf16, name="Eout")

            # ---------------- startup ----------------
            wsrc = persist.tile([1, 256], bf16, name="wsrc")
            nc.vector.memset(wsrc[:], 0.0)
            actd = persist.tile([1, 1], f32, name="actd")
            nc.vector.memset(actd[:], 1.0)
            nc.scalar.sqrt(actd[:], actd[:])  # preload the Sqrt ACT table
            # PE pstate warmup: keep TensorE busy from ~t=1us so the row-0
            # matmuls dispatch against a warm ramp instead of cold pstate
            nc.sync.dma_start(gate[:], gate_d[:])
            nc.scalar.dma_start(lhsT[:], lhst_d[:])
            nc.gpsimd.memset(inf2[:], INF)
            nc.vector.memset(E0[:, 0:1], INF)
            nc.vector.memset(E1[:, 0:1], INF)

            # ---------------- main loop over DP rows ----------------
            for j in range(NL):
                d3 = dist_pool.tile([128, 2 * NT], f32, name="d3")
                ps = psum_pool.tile([128, 2 * NT], f32, name="ps")
                if j == 0:
                    # PE pstate warmup: keep TensorE busy from ~t=1us so
                    # the row-0 matmuls dispatch against a warm ramp
                    for _ in range(11):
                        nc.tensor.matmul(
                            ps[0:1, 0:256], wsrc[:, 0:1], wsrc[:],
                            start=True, stop=True, skip_group_check=True,
                        )
                for hh in range(2):
                    for q in range(2):
                        c0 = hh * NT + q * (NT // 2)
                        nc.tensor.matmul(
                            ps[:, c0 : c0 + NT // 2],
                            lhsT01[:, :, j] if j < 2 else lhsT[:, :, j],
                            xw[hh][:, q * (NT // 2) : (q + 1) * (NT // 2)],
                            start=True,
                            stop=True,
                        )
                    # per-half d' sqrt: finer-grained deps (scan_h
                    # waits only its own half) and keeps the tc
                    # scheduler from hoisting a 2048-wide op between
                    # another row's halves
                    nc.scalar.sqrt(
                        d3[:, hh * NT : (hh + 1) * NT],
                        ps[:, hh * NT : (hh + 1) * NT],
                    )

                Ecur, Eprev = E[j % 2], E[(j + 1) % 2]
                if j == 0:
                    # cumsum per half (separate scans: state must reset
                    # to 0 at the h1 start, no min-protection on row 0)
                    for hh in range(2):
                        nc.vector.tensor_tensor_scan(
                            out=Ecur[:, hh * NT + 1 : (hh + 1) * NT + 1],
                            data0=inf2[:],
                            data1=d3[:, hh * NT : (hh + 1) * NT],
                            initial=0.0,
                            op0=AOT.min,
                            op1=AOT.add,
                        )
                    # row 0 is monotone in t: row 1's window-min is the
                    # shifted row.  Patch the two positions the shifted
                    # view gets wrong: slot 0 <- E_h0[0] (edge) and slot
                    # NT <- E_h1[0] (E_h0[NT-1] there is never needed:
                    # by monotonicity position NT-1 reads slot NT-1).
                    nc.vector.tensor_copy(
                        out=Ecur[:, 0:1], in_=Ecur[:, 1:2]
                    )
                    nc.vector.tensor_copy(
                        out=Ecur[:, NT : NT + 1], in_=Ecur[:, NT + 1 : NT + 2]
                    )
                elif j == 1:
                    # min(E0[t], E0[t-1]) == E0[t-1] by monotonicity:
                    # the shifted row IS the window-min, no TT op
                    for hh in range(2):
                        nc.vector.tensor_tensor_scan(
                            out=Ecur[:, hh * NT + 1 : (hh + 1) * NT + 1],
                            data0=Eprev[:, hh * NT : (hh + 1) * NT],
                            data1=d3[:, hh * NT : (hh + 1) * NT],
                            initial=INF,
                            op0=AOT.min,
                            op1=AOT.add,
                        )
                    # restore the INF edge for later rows reusing E0
                    nc.vector.memset(Eprev[:, 0:1], INF)
                else:
                    # Window-min in bf16 (2x_1p DVE mode, half cost),
                    # split per half-group and interleaved with the
                    # scans so every same-engine dependency has a full
                    # op of slack (no semaphore bubbles on DVE):
                    #   wmin_h0, wmin_h1, scan_h0, scan_h1
                    c3 = c_pool.tile([128, 2 * NT], bf16, name="c3")
                    for hh in range(2):
                        nc.vector.tensor_tensor(
                            c3[:, hh * NT : (hh + 1) * NT],
                            Eprev[:, hh * NT + 1 : (hh + 1) * NT + 1],
                            Eprev[:, hh * NT : (hh + 1) * NT],
                            op=AOT.min,
                        )
                    if j < NL - 1:
                        for hh in range(2):
                            nc.vector.tensor_tensor_scan(
                                out=Ecur[:, hh * NT + 1 : (hh + 1) * NT + 1],
                                data0=c3[:, hh * NT : (hh + 1) * NT],
                                data1=d3[:, hh * NT : (hh + 1) * NT],
                                initial=INF,
                                op0=AOT.min,
                                op1=AOT.add,
                            )
                    else:
                        # final row: chunked scans + eager DMAs so only
                        # the last (short) chunk's DMA sits in the tail
                        chunks = [(0, 0, 1024), (1, 0, 768), (1, 768, 256)]
                        for ci, (hh, q0, ln) in enumerate(chunks):
                            s0 = hh * NT + q0
                            nc.vector.tensor_tensor_scan(
                                out=Eout[:, s0 : s0 + ln],
                                data0=c3[:, s0 : s0 + ln],
                                data1=d3[:, s0 : s0 + ln],
                                initial=(INF if q0 == 0
                                         else Eout[:, s0 - 1 : s0]),
                                op0=AOT.min,
                                op1=AOT.add,
                            )
                            eng = (nc.sync, nc.scalar)[ci % 2]
                            eng.dma_start(
                                out_d[hh][:, q0 : q0 + ln],
                                Eout[:, s0 : s0 + ln],
                            )



    nc.compile()
    _CACHE[key] = nc
    return nc


def _in_maps(x, patts):
    w2inv, _ = _tables()
    x = np.asarray(x, dtype=np.float32)
    patts = np.asarray(patts, dtype=np.float32)

    # lhsT [K, 128, NL]: rows 0..11 block-diag patts (bq*3+d, bq*32+p),
    # rows 12..15 per-b x2 indicators, row 16 = p2 + eps
    lhst = np.zeros((K, 128, NL), np.float32)
    for bq in range(4):
        for d in range(ND):
            lhst[bq * 3 + d, bq * 32 : (bq + 1) * 32, :] = patts[:, d, :]
        lhst[12 + bq, bq * 32 : (bq + 1) * 32, :] = 1.0
    p2e = (patts ** 2).sum(axis=1) + EPS          # (P, NL)
    lhst[16, :, :] = np.tile(p2e, (4, 1))

    maps = []
    for c in range(NCORES):
        xb = x[c * BPC : (c + 1) * BPC]           # (8, 3, NT)
        x2 = (xb ** 2).sum(axis=1)                # (8, NT)
        xws = []
        for h in range(2):
            xwh = np.empty((K, NT), np.float32)
            for bq in range(4):
                b = h * 4 + bq
                xwh[bq * 3 : bq * 3 + 3] = xb[b] * (-2.0 * w2inv)[None, :]
                xwh[12 + bq] = x2[b] * w2inv
            xwh[16] = w2inv
            xws.append(np.ascontiguousarray(xwh))
        maps.append(
            {
                "lhst": np.ascontiguousarray(lhst),
                "gate": np.ascontiguousarray(
                    np.concatenate(
                        [xws[0], xws[1], lhst[:, :, 0:2].reshape(K, 256)],
                        axis=1,
                    )
                ),
            }
        )
    return maps


def _post(res):
    _, wpos = _tables()
    outs = []
    for r in res.results:
        e = np.asarray(r["eout"], dtype=np.float32)   # (2, 128, NT)
        outs.append(e.reshape(BPC, NP, NT) * wpos[None, None, :])
    return np.concatenate(outs, axis=0).astype(np.float32)


def kernel(x, patts):
    nc = _build()
    from concourse.bass_utils import run_bass_kernel_spmd

    res = run_bass_kernel_spmd(
        nc, _in_maps(x, patts), core_ids=list(range(NCORES))
    )
    _CACHE["last_results"] = res
    return _post(res)
